# revision 17
# baseline (speedup 1.0000x reference)
"""Trainium2 Bass kernel for DecoderLinear_for_EffectiveLP_multiclass (v3).

Math (reference):
    src = x @ w_src.T + b_src            # [N]
    dst = x @ w_dst.T + b_dst            # [N]
    s_ij = sigmoid(src[i] + dst[j])      # [N, N]
    channels: p_nb=(1-s_ij)(1-s_ji), p_pu=s_ij(1-s_ji),
              p_pb=s_ij*s_ji,        p_nu=(1-s_ij)s_ji
    out = log(clip(probs, 1e-10, 1))     # [N*N, 4]

Identity with one shared log (sp = softplus):
    L = sp(z1) + sp(z2) = ln((1+e^z1)(1+e^z2))
    ch0 = -L; ch1 = z1-L; ch3 = z2-L; ch2 = z1+z2-L
so ACT does 3 element passes per (i,j): Exp(z1), Exp(z2), Ln(u),
with u = (1+e1)(1+e2) built on DVE from cheap TS/TT ops.

v3 vs v2 (148 us):
  - Projections moved to PE: the host supplies xT (x transposed) and
    replicated weight tiles wb; ps[p,j] = sum_d w[d] xT[d,j] gives every
    partition the projected row directly (row-broadcast for free).  This
    removes the 72 DVE reduction ops, PE transposes and selector matmuls.
  - One Ln per tile instead of two (shared-log identity above).
  - All channel math is TS/TT class (fp16 2x/4x DVE modes); no STT in the
    main loop (STT has no fast fp16 uop: measured 1103ns vs TT 690ns).
  - Input DMAs issue from the gpsimd queue; the sync queue carries only
    the output stream (16.8 MB fp16 per core).
Layout: [rows, ch, j] fp16 in HBM; host reassembles/upcasts to [N*N,4] f32.
"""

import numpy as np

import concourse.bass as bass
import concourse.mybir as mybir
from concourse.bass_utils import run_bass_kernel_spmd

N = 4096
D = 256
NCORES = 8
P = 128
RPC = N // NCORES   # 512 rows per core
RB = RPC // P       # 4 row-blocks per core
KC = D // P         # 2 contraction chunks
TJ = 2048           # j-tile width of the main loop
NJC = N // TJ       # 2 j-tiles
NIT = RB * NJC      # 8 main-loop iterations (jc-major)
CW = 1024           # bcast production chunk width
NCH = N // CW       # 4 production chunks
NBE = 2             # e/u/L double-buffer depth
NBO = 3             # out tile buffer depth

F32 = mybir.dt.float32
F16 = mybir.dt.float16
ALU = mybir.AluOpType
ACTF = mybir.ActivationFunctionType

# gpsimd offload of z2t/p0/p3 was tried and yields NaNs on this build
# (gpsimd fp16 tensor_tensor/tensor_scalar ucode); keep channel math on DVE.
GP_OFFLOAD = False

_compiled = {}


def _build_nc():
    nc = bass.Bass("TRN2")

    xT_d = nc.declare_dram_parameter("xT", [D, N], F16, isOutput=False)
    xb_d = nc.declare_dram_parameter("xb", [RPC, D], F16, isOutput=False)
    wb_d = nc.declare_dram_parameter("wb", [2 * KC * P, P], F16, isOutput=False)
    w2_d = nc.declare_dram_parameter("w2", [2, D], F16, isOutput=False)
    bb_d = nc.declare_dram_parameter("bb", [1, 1], F32, isOutput=False)
    out_d = nc.declare_dram_parameter("out", [RPC, 4 * N], F16, isOutput=True)
    out_d3 = out_d[:].rearrange("r (c n) -> r c n", c=4)
    xT_blocked = xT_d[:].rearrange("(kc d) j -> d kc j", d=P)   # [128, 2, 4096]
    xb_blocked = xb_d[:].rearrange("(b p) d -> p b d", p=P)     # [128, 4, 256]
    wb_blocked = wb_d[:].rearrange("(t d) p -> d t p", d=P)     # [128, 4, 128]

    from contextlib import ExitStack

    with ExitStack() as ctx:
        ec = ctx.enter_context
        # SBUF
        xT_sb = ec(nc.sbuf_tensor("xT_sb", [P, KC * N], F16))
        xT_sb3 = xT_sb[:].rearrange("d (kc j) -> d kc j", kc=KC)
        xb_sb = ec(nc.sbuf_tensor("xb_sb", [P, RB * D], F16))
        xb_sb3 = xb_sb[:].rearrange("p (b d) -> p b d", d=D)
        wb_sb = ec(nc.sbuf_tensor("wb_sb", [P, 2 * KC * P], F16))
        wb_sb3 = wb_sb[:].rearrange("d (t p) -> d t p", p=P)
        w_src_b = ec(nc.sbuf_tensor("w_src_b", [P, D], F16))
        w_dst_b = ec(nc.sbuf_tensor("w_dst_b", [P, D], F16))
        bsum_col = ec(nc.sbuf_tensor("bsum_col", [P, 1], F32))
        bias_src = ec(nc.sbuf_tensor("bias_src", [P, RB], F32))
        bias_dst = ec(nc.sbuf_tensor("bias_dst", [P, RB], F32))
        junk = ec(nc.sbuf_tensor("junk", [P, D], F16))
        s_bcast = ec(nc.sbuf_tensor("s_bcast", [P, N], F16))
        d_bcast = ec(nc.sbuf_tensor("d_bcast", [P, N], F16))
        e1 = [ec(nc.sbuf_tensor(f"e1_{i}", [P, TJ], F16)) for i in range(NBE)]
        e2 = [ec(nc.sbuf_tensor(f"e2_{i}", [P, TJ], F16)) for i in range(NBE)]
        v1 = ec(nc.sbuf_tensor("v1", [P, TJ], F16))
        v2 = ec(nc.sbuf_tensor("v2", [P, TJ], F16))
        ub = [ec(nc.sbuf_tensor(f"ub{i}", [P, TJ], F16)) for i in range(NBE)]
        Lb = [ec(nc.sbuf_tensor(f"Lb{i}", [P, TJ], F16)) for i in range(NBE)]
        z1t = ec(nc.sbuf_tensor("z1t", [P, TJ], F16))
        z2t = [ec(nc.sbuf_tensor(f"z2t{i}", [P, TJ], F16)) for i in range(NBE)]
        outb = [
            ec(nc.sbuf_tensor(f"outb{i}", [P, 4 * TJ], F16)) for i in range(NBO)
        ]
        # PSUM: two chunk buffers [src 1024 | dst 1024] each
        ps = [ec(nc.psum_tensor(f"ps{i}", [P, 2 * CW], F32)) for i in range(2)]
        # semaphores
        s_w = ec(nc.semaphore("s_w"))
        s_xb = ec(nc.semaphore("s_xb"))
        s_wb = ec(nc.semaphore("s_wb"))
        s_x = [ec(nc.semaphore(f"s_x{c}")) for c in range(NCH)]
        s_pe = ec(nc.semaphore("s_pe"))
        s_bccp = ec(nc.semaphore("s_bccp"))
        s_bias = ec(nc.semaphore("s_bias"))
        s_e = ec(nc.semaphore("s_e"))
        s_u = ec(nc.semaphore("s_u"))
        s_l = ec(nc.semaphore("s_l"))
        s_z2 = ec(nc.semaphore("s_z2"))
        s_dve = ec(nc.semaphore("s_dve"))
        s_gp = ec(nc.semaphore("s_gp"))
        s_out = ec(nc.semaphore("s_out"))

        def mm(out_ap, lh, rh, start, stop):
            return nc.tensor.matmul(out_ap, lh, rh, start=start, stop=stop)

        with nc.Block() as block:

            @block.tensor
            def _(t):
                t.wait_ge(s_wb, 16)
                for c in range(NCH):
                    t.wait_ge(s_x[c], 16)
                    if c >= 2:
                        t.wait_ge(s_bccp, c - 1)
                    pst = ps[c % 2]
                    ins = None
                    for proj in range(2):          # 0=src, 1=dst
                        for kc in range(KC):
                            for fh in range(2):    # 512-wide PSUM bank halves
                                ins = mm(
                                    pst[
                                        :,
                                        proj * CW + fh * 512 : proj * CW
                                        + (fh + 1) * 512,
                                    ],
                                    wb_sb3[:, proj * KC + kc, :],
                                    xT_sb3[
                                        :, kc, c * CW + fh * 512 : c * CW
                                        + (fh + 1) * 512
                                    ],
                                    kc == 0,
                                    kc == KC - 1,
                                )
                    ins.then_inc(s_pe, 1)

            @block.scalar
            def _(s):
                def copies(c):
                    s.wait_ge(s_pe, c + 1)
                    csl = slice(c * CW, (c + 1) * CW)
                    nc.scalar.copy(s_bcast[:, csl], ps[c % 2][:, 0:CW])
                    nc.scalar.copy(
                        d_bcast[:, csl], ps[c % 2][:, CW : 2 * CW]
                    ).then_inc(s_bccp, 1)

                def exps(it):
                    jc, rb = divmod(it, RB)
                    b = it % NBE
                    jsl = slice(jc * TJ, (jc + 1) * TJ)
                    bs = bias_src[:, rb : rb + 1]
                    bd = bias_dst[:, rb : rb + 1]
                    if it >= NBE:
                        s.wait_ge(s_u, it - NBE + 1)
                    nc.scalar.activation(
                        e1[b][:], d_bcast[:, jsl], ACTF.Exp, bias=bs, scale=1.0
                    )
                    nc.scalar.activation(
                        e2[b][:], s_bcast[:, jsl], ACTF.Exp, bias=bd, scale=1.0
                    ).then_inc(s_e, 1)

                copies(0)
                copies(1)
                s.wait_ge(s_bias, 1)
                exps(0)
                exps(1)
                for it in range(NIT):
                    b = it % NBE
                    s.wait_ge(s_u, it + 1)
                    if it >= NBE:
                        s.wait_ge(s_dve, 2 * (it - NBE + 1))
                        if GP_OFFLOAD:
                            s.wait_ge(s_gp, it - NBE + 1)
                    nc.scalar.activation(
                        Lb[b][:], ub[b][:], ACTF.Ln, bias=0.0, scale=1.0
                    ).then_inc(s_l, 1)
                    if it == 2:
                        copies(2)
                        copies(3)
                    if it + 2 < NIT:
                        exps(it + 2)

            @block.vector
            def _(v):
                # per-core bias columns from this core's own rows
                v.wait_ge(s_w, 48)  # bsum + w_src_b + w_dst_b
                v.wait_ge(s_xb, 16)
                for rb in range(RB):
                    xt = xb_sb3[:, rb, :]
                    nc.vector.scalar_tensor_tensor(
                        out=junk[:], in0=xt, scalar=1.0, in1=w_src_b[:],
                        op0=ALU.mult, op1=ALU.mult,
                        accum_out=bias_src[:, rb : rb + 1],
                    )
                    nc.vector.scalar_tensor_tensor(
                        out=junk[:], in0=xt, scalar=1.0, in1=w_dst_b[:],
                        op0=ALU.mult, op1=ALU.mult,
                        accum_out=bias_dst[:, rb : rb + 1],
                    )
                nc.vector.tensor_scalar(
                    out=bias_src[:], in0=bias_src[:],
                    scalar1=bsum_col[:, 0:1], scalar2=None, op0=ALU.add,
                )
                nc.vector.tensor_scalar(
                    out=bias_dst[:], in0=bias_dst[:],
                    scalar1=bsum_col[:, 0:1], scalar2=None, op0=ALU.add,
                ).then_inc(s_bias, 1)
                for it in range(NIT):
                    jc, rb = divmod(it, RB)
                    b, o = it % NBE, it % NBO
                    jsl = slice(jc * TJ, (jc + 1) * TJ)
                    bs = bias_src[:, rb : rb + 1]
                    bd = bias_dst[:, rb : rb + 1]
                    v.wait_ge(s_e, it + 1)
                    if it >= NBE:
                        v.wait_ge(s_l, it - NBE + 1)
                    nc.vector.tensor_scalar(
                        out=v1[:], in0=e1[b][:], scalar1=1.0, scalar2=None,
                        op0=ALU.add,
                    )
                    nc.vector.tensor_scalar(
                        out=v2[:], in0=e2[b][:], scalar1=1.0, scalar2=None,
                        op0=ALU.add,
                    )
                    nc.vector.tensor_tensor(
                        out=ub[b][:], in0=v1[:], in1=v2[:], op=ALU.mult
                    ).then_inc(s_u, 1)
                    # z tiles while ACT computes L
                    nc.vector.tensor_scalar(
                        out=z1t[:], in0=d_bcast[:, jsl], scalar1=bs,
                        scalar2=None, op0=ALU.add,
                    )
                    if not GP_OFFLOAD:
                        nc.vector.tensor_scalar(
                            out=z2t[b][:], in0=s_bcast[:, jsl], scalar1=bd,
                            scalar2=None, op0=ALU.add,
                        )
                    v.wait_ge(s_l, it + 1)
                    if it >= NBO:
                        v.wait_ge(s_out, 32 * (it - NBO + 1))
                    ot = outb[o]
                    p0 = ot[:, 0:TJ]
                    p1 = ot[:, TJ : 2 * TJ]
                    p2 = ot[:, 2 * TJ : 3 * TJ]
                    p3 = ot[:, 3 * TJ : 4 * TJ]
                    # planes 0+1 first, then 2+3: the out-DMA is split in
                    # half so the first 1 MiB flies while p3/p2 compute
                    nc.vector.tensor_scalar(
                        out=p0, in0=Lb[b][:], scalar1=-1.0, scalar2=None,
                        op0=ALU.mult,
                    )
                    nc.vector.tensor_tensor(
                        out=p1, in0=z1t[:], in1=Lb[b][:], op=ALU.subtract
                    ).then_inc(s_dve, 1)
                    nc.vector.tensor_tensor(
                        out=p3, in0=z2t[b][:], in1=Lb[b][:], op=ALU.subtract
                    )
                    nc.vector.tensor_tensor(
                        out=p2, in0=z2t[b][:], in1=p1, op=ALU.add
                    ).then_inc(s_dve, 1)

            if GP_OFFLOAD:

                @block.gpsimd
                def _(g):
                    g.wait_ge(s_bias, 1)
                    for it in range(NIT):
                        jc, rb = divmod(it, RB)
                        b, o = it % NBE, it % NBO
                        jsl = slice(jc * TJ, (jc + 1) * TJ)
                        bd = bias_dst[:, rb : rb + 1]
                        g.wait_ge(s_bccp, 2 * (jc + 1))
                        if it >= NBE:
                            g.wait_ge(s_dve, it - NBE + 1)
                        nc.gpsimd.tensor_scalar(
                            out=z2t[b][:], in0=s_bcast[:, jsl], scalar1=bd,
                            scalar2=None, op0=ALU.add,
                        ).then_inc(s_z2, 1)
                        g.wait_ge(s_l, it + 1)
                        if it >= NBO:
                            g.wait_ge(s_out, 16 * (it - NBO + 1))
                        ot = outb[o]
                        p0 = ot[:, 0:TJ]
                        p3 = ot[:, 3 * TJ : 4 * TJ]
                        nc.gpsimd.tensor_scalar(
                            out=p0, in0=Lb[b][:], scalar1=-1.0, scalar2=None,
                            op0=ALU.mult,
                        )
                        nc.gpsimd.tensor_tensor(
                            out=p3, in0=z2t[b][:], in1=Lb[b][:],
                            op=ALU.subtract,
                        ).then_inc(s_gp, 1)

            @block.sync
            def _(sy):
                # input DMAs first, in critical-path order: wb + xT c0 gate
                # PE; bias-path inputs gate the first exps; bulk xT last.
                # Per-chunk semaphores (s_x[c]) so a chunk's readiness can
                # never be faked by packet completions of a later chunk.
                # All inputs (2.6 MB) drain well before the first out-DMA
                # needs the queue at ~12 us.
                sy.dma_start(out=wb_sb3[:, :, :], in_=wb_blocked[:, :, :]).then_inc(
                    s_wb, 16
                )
                sy.dma_start(
                    out=xT_sb3[:, :, 0:CW], in_=xT_blocked[:, :, 0:CW]
                ).then_inc(s_x[0], 16)
                sy.dma_start(
                    out=bsum_col[:],
                    in_=bb_d[0:1, :].partition_broadcast(P)[:, 0, :],
                ).then_inc(s_w, 16)
                sy.dma_start(
                    out=w_src_b[:],
                    in_=w2_d[0:1, :].partition_broadcast(P)[:, 0, :],
                ).then_inc(s_w, 16)
                sy.dma_start(
                    out=w_dst_b[:],
                    in_=w2_d[1:2, :].partition_broadcast(P)[:, 0, :],
                ).then_inc(s_w, 16)
                sy.dma_start(out=xb_sb3[:, :, :], in_=xb_blocked[:, :, :]).then_inc(
                    s_xb, 16
                )
                for c in range(1, NCH):
                    sy.dma_start(
                        out=xT_sb3[:, :, c * CW : (c + 1) * CW],
                        in_=xT_blocked[:, :, c * CW : (c + 1) * CW],
                    ).then_inc(s_x[c], 16)
                for it in range(NIT):
                    jc, rb = divmod(it, RB)
                    o = it % NBO
                    rsl = slice(rb * P, (rb + 1) * P)
                    jsl = slice(jc * TJ, (jc + 1) * TJ)
                    ob3 = outb[o][:].rearrange("p (c n) -> p c n", c=4)
                    sy.wait_ge(s_dve, 2 * it + 1)
                    sy.dma_start(
                        out=out_d3[rsl, 0:2, jsl], in_=ob3[:, 0:2, :]
                    ).then_inc(s_out, 16)
                    sy.wait_ge(s_dve, 2 * it + 2)
                    sy.dma_start(
                        out=out_d3[rsl, 2:4, jsl], in_=ob3[:, 2:4, :]
                    ).then_inc(s_out, 16)
                sy.wait_ge(s_out, 32 * NIT)

    return nc


def _get_nc():
    if "nc" not in _compiled:
        _compiled["nc"] = _build_nc()
    return _compiled["nc"]


def _make_in_maps(inputs):
    x = np.asarray(inputs["x"], dtype=np.float32)
    w_src = np.asarray(inputs["w_src"], dtype=np.float32).reshape(D)
    w_dst = np.asarray(inputs["w_dst"], dtype=np.float32).reshape(D)
    b_src = np.asarray(inputs["b_src"], dtype=np.float32).reshape(-1)[0]
    b_dst = np.asarray(inputs["b_dst"], dtype=np.float32).reshape(-1)[0]
    x16 = x.astype(np.float16)
    xT = np.ascontiguousarray(x16.T)                     # [D, N]
    # wb rows: src kc0 | src kc1 | dst kc0 | dst kc1, each [128, 128]
    tiles = []
    for w in (w_src, w_dst):
        for kc in range(KC):
            seg = w[kc * P : (kc + 1) * P].astype(np.float16)
            tiles.append(np.repeat(seg[:, None], P, axis=1))
    wb = np.ascontiguousarray(np.concatenate(tiles, axis=0))  # [512, 128]
    bb = np.array([[np.float32(b_src) + np.float32(b_dst)]], dtype=np.float32)
    in_maps = []
    w2 = np.ascontiguousarray(
        np.stack([w_src, w_dst], axis=0).astype(np.float16)
    )
    for m in range(NCORES):
        xb = np.ascontiguousarray(x16[m * RPC : (m + 1) * RPC, :])
        in_maps.append({"xT": xT, "xb": xb, "wb": wb, "w2": w2, "bb": bb})
    return in_maps


def _assemble(results):
    full = np.empty((N, N, 4), dtype=np.float32)
    for m in range(NCORES):
        blk = results[m]["out"].reshape(RPC, 4, N)
        full[m * RPC : (m + 1) * RPC] = blk.transpose(0, 2, 1)
    return full.reshape(N * N, 4)


def kernel(**inputs) -> np.ndarray:
    nc = _get_nc()
    res = run_bass_kernel_spmd(nc, _make_in_maps(inputs), core_ids=list(range(NCORES)))
    return _assemble(res.results)


def kernel_traced(**inputs):
    """Like kernel() but also returns (output, exec_time_ns, profile_json)."""
    nc = _get_nc()
    res = run_bass_kernel_spmd(
        nc, _make_in_maps(inputs), core_ids=list(range(NCORES)), trace=True
    )
    return _assemble(res.results), res.exec_time_ns, res.profile_json


# revision 20
# speedup vs baseline: 1.1325x; 1.1325x over previous
"""Trainium2 Bass kernel for DecoderLinear_for_EffectiveLP_multiclass (v3).

Math (reference):
    src = x @ w_src.T + b_src            # [N]
    dst = x @ w_dst.T + b_dst            # [N]
    s_ij = sigmoid(src[i] + dst[j])      # [N, N]
    channels: p_nb=(1-s_ij)(1-s_ji), p_pu=s_ij(1-s_ji),
              p_pb=s_ij*s_ji,        p_nu=(1-s_ij)s_ji
    out = log(clip(probs, 1e-10, 1))     # [N*N, 4]

Identity with one shared log (sp = softplus):
    L = sp(z1) + sp(z2) = ln((1+e^z1)(1+e^z2))
    ch0 = -L; ch1 = z1-L; ch3 = z2-L; ch2 = z1+z2-L
so ACT does 3 element passes per (i,j): Exp(z1), Exp(z2), Ln(u),
with u = (1+e1)(1+e2) built on DVE from cheap TS/TT ops.

v3 vs v2 (148 us):
  - Projections moved to PE: the host supplies xT (x transposed) and
    replicated weight tiles wb; ps[p,j] = sum_d w[d] xT[d,j] gives every
    partition the projected row directly (row-broadcast for free).  This
    removes the 72 DVE reduction ops, PE transposes and selector matmuls.
  - One Ln per tile instead of two (shared-log identity above).
  - All channel math is TS/TT class (fp16 2x/4x DVE modes); no STT in the
    main loop (STT has no fast fp16 uop: measured 1103ns vs TT 690ns).
  - Input DMAs issue from the gpsimd queue; the sync queue carries only
    the output stream (16.8 MB fp16 per core).
Layout: [rows, ch, j] fp16 in HBM; host reassembles/upcasts to [N*N,4] f32.
"""

import numpy as np

import concourse.bass as bass
import concourse.mybir as mybir
from concourse.bass_utils import run_bass_kernel_spmd

N = 4096
D = 256
NCORES = 8
P = 128
RPC = N // NCORES   # 512 rows per core
RB = RPC // P       # 4 row-blocks per core
KC = D // P         # 2 contraction chunks
TJ = 2048           # j-tile width of the main loop
NJC = N // TJ       # 2 j-tiles
NIT = RB * NJC      # 8 main-loop iterations (jc-major)
CW = 1024           # bcast production chunk width
NCH = N // CW       # 4 production chunks
NBE = 2             # e/u/L double-buffer depth
NBO = 3             # out tile buffer depth

F32 = mybir.dt.float32
F16 = mybir.dt.float16
ALU = mybir.AluOpType
ACTF = mybir.ActivationFunctionType

# gpsimd offload of z2t/p0/p3 was tried and yields NaNs on this build
# (gpsimd fp16 tensor_tensor/tensor_scalar ucode); keep channel math on DVE.
GP_OFFLOAD = False

_compiled = {}


def _build_nc():
    nc = bass.Bass("TRN2")

    xT_d = nc.declare_dram_parameter("xT", [D, N], F16, isOutput=False)
    xb_d = nc.declare_dram_parameter("xb", [RPC, D], F16, isOutput=False)
    wb_d = nc.declare_dram_parameter("wb", [2 * KC * P, P], F16, isOutput=False)
    w2_d = nc.declare_dram_parameter("w2", [2, D], F16, isOutput=False)
    bb_d = nc.declare_dram_parameter("bb", [1, 1], F32, isOutput=False)
    out_d = nc.declare_dram_parameter("out", [RPC, 4 * N], F16, isOutput=True)
    out_d3 = out_d[:].rearrange("r (c n) -> r c n", c=4)
    xT_blocked = xT_d[:].rearrange("(kc d) j -> d kc j", d=P)   # [128, 2, 4096]
    xb_blocked = xb_d[:].rearrange("(b p) d -> p b d", p=P)     # [128, 4, 256]
    wb_blocked = wb_d[:].rearrange("(t d) p -> d t p", d=P)     # [128, 4, 128]

    from contextlib import ExitStack

    with ExitStack() as ctx:
        ec = ctx.enter_context
        # SBUF
        xT_sb = ec(nc.sbuf_tensor("xT_sb", [P, KC * N], F16))
        xT_sb3 = xT_sb[:].rearrange("d (kc j) -> d kc j", kc=KC)
        xb_sb = ec(nc.sbuf_tensor("xb_sb", [P, RB * D], F16))
        xb_sb3 = xb_sb[:].rearrange("p (b d) -> p b d", d=D)
        wb_sb = ec(nc.sbuf_tensor("wb_sb", [P, 2 * KC * P], F16))
        wb_sb3 = wb_sb[:].rearrange("d (t p) -> d t p", p=P)
        w_src_b = ec(nc.sbuf_tensor("w_src_b", [P, D], F16))
        w_dst_b = ec(nc.sbuf_tensor("w_dst_b", [P, D], F16))
        bsum_col = ec(nc.sbuf_tensor("bsum_col", [P, 1], F32))
        bias_src = ec(nc.sbuf_tensor("bias_src", [P, RB], F32))
        bias_dst = ec(nc.sbuf_tensor("bias_dst", [P, RB], F32))
        junk = ec(nc.sbuf_tensor("junk", [P, D], F16))
        s_bcast = ec(nc.sbuf_tensor("s_bcast", [P, N], F16))
        d_bcast = ec(nc.sbuf_tensor("d_bcast", [P, N], F16))
        e1 = [ec(nc.sbuf_tensor(f"e1_{i}", [P, TJ], F16)) for i in range(NBE)]
        e2 = [ec(nc.sbuf_tensor(f"e2_{i}", [P, TJ], F16)) for i in range(NBE)]
        v1 = ec(nc.sbuf_tensor("v1", [P, TJ], F16))
        v2 = ec(nc.sbuf_tensor("v2", [P, TJ], F16))
        ub = [ec(nc.sbuf_tensor(f"ub{i}", [P, TJ], F16)) for i in range(NBE)]
        Lb = [ec(nc.sbuf_tensor(f"Lb{i}", [P, TJ], F16)) for i in range(NBE)]
        z1t = ec(nc.sbuf_tensor("z1t", [P, TJ], F16))
        z2t = [ec(nc.sbuf_tensor(f"z2t{i}", [P, TJ], F16)) for i in range(NBE)]
        outb = [
            ec(nc.sbuf_tensor(f"outb{i}", [P, 4 * TJ], F16)) for i in range(NBO)
        ]
        # PSUM: two chunk buffers [src 1024 | dst 1024] each
        ps = [ec(nc.psum_tensor(f"ps{i}", [P, 2 * CW], F32)) for i in range(2)]
        # semaphores
        s_w = ec(nc.semaphore("s_w"))
        s_xb = ec(nc.semaphore("s_xb"))
        s_wb = ec(nc.semaphore("s_wb"))
        s_x = [ec(nc.semaphore(f"s_x{c}")) for c in range(NCH)]
        s_pe = ec(nc.semaphore("s_pe"))
        s_bccp = ec(nc.semaphore("s_bccp"))
        s_bias = ec(nc.semaphore("s_bias"))
        s_e = ec(nc.semaphore("s_e"))
        s_u = ec(nc.semaphore("s_u"))
        s_l = ec(nc.semaphore("s_l"))
        s_z2 = ec(nc.semaphore("s_z2"))
        s_dve = ec(nc.semaphore("s_dve"))
        s_gp = ec(nc.semaphore("s_gp"))
        s_out = ec(nc.semaphore("s_out"))

        def mm(out_ap, lh, rh, start, stop):
            return nc.tensor.matmul(out_ap, lh, rh, start=start, stop=stop)

        with nc.Block() as block:

            @block.tensor
            def _(t):
                t.wait_ge(s_wb, 16)
                for c in range(NCH):
                    t.wait_ge(s_x[c], 16)
                    if c >= 2:
                        t.wait_ge(s_bccp, c - 1)
                    pst = ps[c % 2]
                    ins = None
                    for proj in range(2):          # 0=src, 1=dst
                        for kc in range(KC):
                            for fh in range(2):    # 512-wide PSUM bank halves
                                ins = mm(
                                    pst[
                                        :,
                                        proj * CW + fh * 512 : proj * CW
                                        + (fh + 1) * 512,
                                    ],
                                    wb_sb3[:, proj * KC + kc, :],
                                    xT_sb3[
                                        :, kc, c * CW + fh * 512 : c * CW
                                        + (fh + 1) * 512
                                    ],
                                    kc == 0,
                                    kc == KC - 1,
                                )
                    ins.then_inc(s_pe, 1)

            @block.scalar
            def _(s):
                def copies(c):
                    s.wait_ge(s_pe, c + 1)
                    csl = slice(c * CW, (c + 1) * CW)
                    nc.scalar.copy(s_bcast[:, csl], ps[c % 2][:, 0:CW])
                    nc.scalar.copy(
                        d_bcast[:, csl], ps[c % 2][:, CW : 2 * CW]
                    ).then_inc(s_bccp, 1)

                def exps(it):
                    jc, rb = divmod(it, RB)
                    b = it % NBE
                    jsl = slice(jc * TJ, (jc + 1) * TJ)
                    bs = bias_src[:, rb : rb + 1]
                    bd = bias_dst[:, rb : rb + 1]
                    if it >= NBE:
                        s.wait_ge(s_u, it - NBE + 1)
                    nc.scalar.activation(
                        e1[b][:], d_bcast[:, jsl], ACTF.Exp, bias=bs, scale=1.0
                    )
                    nc.scalar.activation(
                        e2[b][:], s_bcast[:, jsl], ACTF.Exp, bias=bd, scale=1.0
                    ).then_inc(s_e, 1)

                copies(0)
                copies(1)
                s.wait_ge(s_bias, 1)
                exps(0)
                exps(1)
                for it in range(NIT):
                    b = it % NBE
                    s.wait_ge(s_u, it + 1)
                    if it >= NBE:
                        s.wait_ge(s_dve, it - NBE + 1)
                        if GP_OFFLOAD:
                            s.wait_ge(s_gp, it - NBE + 1)
                    nc.scalar.activation(
                        Lb[b][:], ub[b][:], ACTF.Ln, bias=0.0, scale=1.0
                    ).then_inc(s_l, 1)
                    if it == 2:
                        copies(2)
                        copies(3)
                    if it + 2 < NIT:
                        exps(it + 2)

            @block.vector
            def _(v):
                # per-core bias columns from this core's own rows
                v.wait_ge(s_w, 48)  # bsum + w_src_b + w_dst_b
                v.wait_ge(s_xb, 16)
                for rb in range(RB):
                    xt = xb_sb3[:, rb, :]
                    nc.vector.scalar_tensor_tensor(
                        out=junk[:], in0=xt, scalar=1.0, in1=w_src_b[:],
                        op0=ALU.mult, op1=ALU.mult,
                        accum_out=bias_src[:, rb : rb + 1],
                    )
                    nc.vector.scalar_tensor_tensor(
                        out=junk[:], in0=xt, scalar=1.0, in1=w_dst_b[:],
                        op0=ALU.mult, op1=ALU.mult,
                        accum_out=bias_dst[:, rb : rb + 1],
                    )
                nc.vector.tensor_scalar(
                    out=bias_src[:], in0=bias_src[:],
                    scalar1=bsum_col[:, 0:1], scalar2=None, op0=ALU.add,
                )
                nc.vector.tensor_scalar(
                    out=bias_dst[:], in0=bias_dst[:],
                    scalar1=bsum_col[:, 0:1], scalar2=None, op0=ALU.add,
                ).then_inc(s_bias, 1)
                for it in range(NIT):
                    jc, rb = divmod(it, RB)
                    b, o = it % NBE, it % NBO
                    jsl = slice(jc * TJ, (jc + 1) * TJ)
                    bs = bias_src[:, rb : rb + 1]
                    bd = bias_dst[:, rb : rb + 1]
                    v.wait_ge(s_e, it + 1)
                    if it >= NBE:
                        v.wait_ge(s_l, it - NBE + 1)
                    nc.vector.tensor_scalar(
                        out=v1[:], in0=e1[b][:], scalar1=1.0, scalar2=None,
                        op0=ALU.add,
                    )
                    nc.vector.tensor_scalar(
                        out=v2[:], in0=e2[b][:], scalar1=1.0, scalar2=None,
                        op0=ALU.add,
                    )
                    nc.vector.tensor_tensor(
                        out=ub[b][:], in0=v1[:], in1=v2[:], op=ALU.mult
                    ).then_inc(s_u, 1)
                    # z tiles while ACT computes L
                    nc.vector.tensor_scalar(
                        out=z1t[:], in0=d_bcast[:, jsl], scalar1=bs,
                        scalar2=None, op0=ALU.add,
                    )
                    if not GP_OFFLOAD:
                        nc.vector.tensor_scalar(
                            out=z2t[b][:], in0=s_bcast[:, jsl], scalar1=bd,
                            scalar2=None, op0=ALU.add,
                        )
                    v.wait_ge(s_l, it + 1)
                    if it >= NBO:
                        v.wait_ge(s_out, 16 * (it - NBO + 1))
                    ot = outb[o]
                    p0 = ot[:, 0:TJ]
                    p1 = ot[:, TJ : 2 * TJ]
                    p2 = ot[:, 2 * TJ : 3 * TJ]
                    p3 = ot[:, 3 * TJ : 4 * TJ]
                    nc.vector.tensor_tensor(
                        out=p1, in0=z1t[:], in1=Lb[b][:], op=ALU.subtract
                    )
                    nc.vector.tensor_scalar(
                        out=p0, in0=Lb[b][:], scalar1=-1.0, scalar2=None,
                        op0=ALU.mult,
                    )
                    nc.vector.tensor_tensor(
                        out=p3, in0=z2t[b][:], in1=Lb[b][:], op=ALU.subtract
                    )
                    nc.vector.tensor_tensor(
                        out=p2, in0=z2t[b][:], in1=p1, op=ALU.add
                    ).then_inc(s_dve, 1)

            if GP_OFFLOAD:

                @block.gpsimd
                def _(g):
                    g.wait_ge(s_bias, 1)
                    for it in range(NIT):
                        jc, rb = divmod(it, RB)
                        b, o = it % NBE, it % NBO
                        jsl = slice(jc * TJ, (jc + 1) * TJ)
                        bd = bias_dst[:, rb : rb + 1]
                        g.wait_ge(s_bccp, 2 * (jc + 1))
                        if it >= NBE:
                            g.wait_ge(s_dve, it - NBE + 1)
                        nc.gpsimd.tensor_scalar(
                            out=z2t[b][:], in0=s_bcast[:, jsl], scalar1=bd,
                            scalar2=None, op0=ALU.add,
                        ).then_inc(s_z2, 1)
                        g.wait_ge(s_l, it + 1)
                        if it >= NBO:
                            g.wait_ge(s_out, 16 * (it - NBO + 1))
                        ot = outb[o]
                        p0 = ot[:, 0:TJ]
                        p3 = ot[:, 3 * TJ : 4 * TJ]
                        nc.gpsimd.tensor_scalar(
                            out=p0, in0=Lb[b][:], scalar1=-1.0, scalar2=None,
                            op0=ALU.mult,
                        )
                        nc.gpsimd.tensor_tensor(
                            out=p3, in0=z2t[b][:], in1=Lb[b][:],
                            op=ALU.subtract,
                        ).then_inc(s_gp, 1)

            @block.sync
            def _(sy):
                # input DMAs first, in critical-path order: wb + xT c0 gate
                # PE; bias-path inputs gate the first exps; bulk xT last.
                # Per-chunk semaphores (s_x[c]) so a chunk's readiness can
                # never be faked by packet completions of a later chunk.
                # All inputs (2.6 MB) drain well before the first out-DMA
                # needs the queue at ~12 us.
                sy.dma_start(out=wb_sb3[:, :, :], in_=wb_blocked[:, :, :]).then_inc(
                    s_wb, 16
                )
                sy.dma_start(
                    out=xT_sb3[:, :, 0:CW], in_=xT_blocked[:, :, 0:CW]
                ).then_inc(s_x[0], 16)
                sy.dma_start(
                    out=bsum_col[:],
                    in_=bb_d[0:1, :].partition_broadcast(P)[:, 0, :],
                ).then_inc(s_w, 16)
                sy.dma_start(
                    out=w_src_b[:],
                    in_=w2_d[0:1, :].partition_broadcast(P)[:, 0, :],
                ).then_inc(s_w, 16)
                sy.dma_start(
                    out=w_dst_b[:],
                    in_=w2_d[1:2, :].partition_broadcast(P)[:, 0, :],
                ).then_inc(s_w, 16)
                sy.dma_start(out=xb_sb3[:, :, :], in_=xb_blocked[:, :, :]).then_inc(
                    s_xb, 16
                )
                for c in range(1, NCH):
                    sy.dma_start(
                        out=xT_sb3[:, :, c * CW : (c + 1) * CW],
                        in_=xT_blocked[:, :, c * CW : (c + 1) * CW],
                    ).then_inc(s_x[c], 16)
                for it in range(NIT):
                    jc, rb = divmod(it, RB)
                    o = it % NBO
                    sy.wait_ge(s_dve, it + 1)
                    sy.dma_start(
                        out=out_d3[
                            rb * P : (rb + 1) * P, :, jc * TJ : (jc + 1) * TJ
                        ],
                        in_=outb[o][:].rearrange("p (c n) -> p c n", c=4),
                    ).then_inc(s_out, 16)
                sy.wait_ge(s_out, 16 * NIT)

    return nc


def _get_nc():
    if "nc" not in _compiled:
        _compiled["nc"] = _build_nc()
    return _compiled["nc"]


def _make_in_maps(inputs):
    x = np.asarray(inputs["x"], dtype=np.float32)
    w_src = np.asarray(inputs["w_src"], dtype=np.float32).reshape(D)
    w_dst = np.asarray(inputs["w_dst"], dtype=np.float32).reshape(D)
    b_src = np.asarray(inputs["b_src"], dtype=np.float32).reshape(-1)[0]
    b_dst = np.asarray(inputs["b_dst"], dtype=np.float32).reshape(-1)[0]
    x16 = x.astype(np.float16)
    xT = np.ascontiguousarray(x16.T)                     # [D, N]
    # wb rows: src kc0 | src kc1 | dst kc0 | dst kc1, each [128, 128]
    tiles = []
    for w in (w_src, w_dst):
        for kc in range(KC):
            seg = w[kc * P : (kc + 1) * P].astype(np.float16)
            tiles.append(np.repeat(seg[:, None], P, axis=1))
    wb = np.ascontiguousarray(np.concatenate(tiles, axis=0))  # [512, 128]
    bb = np.array([[np.float32(b_src) + np.float32(b_dst)]], dtype=np.float32)
    in_maps = []
    w2 = np.ascontiguousarray(
        np.stack([w_src, w_dst], axis=0).astype(np.float16)
    )
    for m in range(NCORES):
        xb = np.ascontiguousarray(x16[m * RPC : (m + 1) * RPC, :])
        in_maps.append({"xT": xT, "xb": xb, "wb": wb, "w2": w2, "bb": bb})
    return in_maps


def _assemble(results):
    full = np.empty((N, N, 4), dtype=np.float32)
    for m in range(NCORES):
        blk = results[m]["out"].reshape(RPC, 4, N)
        full[m * RPC : (m + 1) * RPC] = blk.transpose(0, 2, 1)
    return full.reshape(N * N, 4)


def kernel(**inputs) -> np.ndarray:
    nc = _get_nc()
    res = run_bass_kernel_spmd(nc, _make_in_maps(inputs), core_ids=list(range(NCORES)))
    return _assemble(res.results)


def kernel_traced(**inputs):
    """Like kernel() but also returns (output, exec_time_ns, profile_json)."""
    nc = _get_nc()
    res = run_bass_kernel_spmd(
        nc, _make_in_maps(inputs), core_ids=list(range(NCORES)), trace=True
    )
    return _assemble(res.results), res.exec_time_ns, res.profile_json
